# revision 55
# baseline (speedup 1.0000x reference)
"""Trainium2 Bass kernel for nn_BallPredictorGNN.

The reference model is a 2-layer GAT over (N=20000, E=640000) followed by an
MLP applied to the LAST node only ("ball") -- the output is a single [2]
vector.  Only the ball's 2-hop dependency cone matters:

  layer 2 aggregates at the ball node only            (~33 in-edges)
  layer 1 aggregates at the ball's in-neighbours S2   (~26 nodes, ~850 edges)
  x @ W1 is needed for the sources of those edges

Host side (pure data routing): extract the cone and lay layer-1 edges out on
a [128 partitions x K chunks] grid; each partition serves one destination
node (high-degree destinations get several partitions).  Source features are
replicated per edge-slot into the xTs operand; destination features go into
a per-partition xTd operand, so a pair of accumulating TensorE matmuls
produces per-edge rows [as+ad | h] directly -- no on-device index math.

Device side: per chunk, ScalarE computes pe = exp(prelu(e)+mask) (both in
the exp table set, mask folds padding to exp(-1e30)=0), VectorE forms
msg = [h*pe | pe], and one TensorE matmul against a host-built one-hot P2
merges partition groups AND gathers layer-2 edge slots in one shot
(agg[q] = sum over layer-1 edges of dst(q)).  Layer 2 then runs fully
on-chip on the [128 edge-slot, .] layout; the softmax division is deferred
through the (relu-positive-homogeneous, zero-bias) MLP and applied as a
single fused Copy(scale=1/den) at the very end.

The same program is replicated SPMD on all 8 NeuronCores (the cone is tiny,
so replication beats sharding + collectives); core 0's output is returned.
"""

import numpy as np

P = 128
NEG = np.float32(-1e30)
_CACHE = {}


def _ceil(a, b):
    return -(-a // b)


class _Packer:
    """Pack many small [p, w] operands into one [128, W] array, column-wise."""

    def __init__(self, dtype):
        self.cols = []
        self.pos = 0
        self.slots = {}
        self.dtype = dtype

    def add(self, name, arr):
        arr = np.asarray(arr, dtype=np.float32)
        p, w = arr.shape
        full = np.zeros((P, w), np.float32)
        full[:p] = arr
        self.cols.append(full)
        self.slots[name] = (self.pos, self.pos + w)
        self.pos += w

    def finish(self):
        return np.ascontiguousarray(
            np.concatenate(self.cols, axis=1).astype(self.dtype)
        )


def _host_preprocess(inputs):
    import ml_dtypes

    x = np.asarray(inputs["x"], dtype=np.float32)
    ei = np.asarray(inputs["edge_index"]).astype(np.int64)
    N, F = x.shape
    ball = N - 1
    src, dst = ei[0], ei[1]

    # ---- layer-2 edges into the ball: self loop FIRST (slot q=0) ------------
    e2s = np.concatenate([[ball], src[dst == ball]]).astype(np.int64)
    n2 = len(e2s)
    assert n2 <= P, f"ball in-neighbourhood too large: {n2}"
    uniq = np.unique(e2s)
    S2 = np.concatenate([[ball], uniq[uniq != ball]]).astype(np.int64)
    m2 = len(S2)
    loc2 = np.full(N, -1, dtype=np.int64)
    loc2[S2] = np.arange(m2)
    s2_loc = loc2[e2s]  # [n2], s2_loc[0] == 0 (ball)

    # ---- layer-1 edge grid: [partition, chunk] ------------------------------
    in_S2 = np.zeros(N, dtype=bool)
    in_S2[S2] = True
    sel1 = in_S2[dst]
    l1s, l1d = src[sel1], loc2[dst[sel1]]
    by_dst = [[v] for v in S2]  # reference adds a self loop to every node
    for s, d in zip(l1s, l1d):
        by_dst[d].append(s)

    K = 2
    while sum(_ceil(len(g), K) for g in by_dst) > P:
        K += 1
    nparts = [_ceil(len(g), K) for g in by_dst]
    assert sum(nparts) <= P

    grid_src = np.zeros((P, K), dtype=np.int64)
    grid_valid = np.zeros((P, K), dtype=bool)
    slotmap = np.full(P, -1, dtype=np.int64)  # partition -> S2 index
    p = 0
    for sidx in range(m2):
        g = by_dst[sidx]
        for gi in range(nparts[sidx]):
            chunk = g[gi * K : (gi + 1) * K]
            for j, s in enumerate(chunk):
                grid_src[p, j] = s
                grid_valid[p, j] = True
            slotmap[p] = sidx
            p += 1

    # xT: [F, (K+1)*128].  Block 0 = destination features per partition;
    # block k+1, column q = x[grid_src[q, k]] (zeroed if pad).
    xd = x[S2[np.maximum(slotmap, 0)]]
    xd[slotmap < 0] = 0
    xg = x[grid_src.T.reshape(-1)]
    xg[~grid_valid.T.reshape(-1)] = 0
    xT = np.ascontiguousarray(
        np.concatenate([xd, xg], axis=0).T.astype(ml_dtypes.bfloat16)
    )

    admask = np.where(grid_valid, np.float32(0), NEG).astype(np.float32)

    # P2: layer-1 aggregation + layer-2 gather one-hot.
    # P2[p, q] = 1 iff partition p serves the dst node of layer-2 edge slot q.
    # Padding slots q>=n2 reuse the ball column (nonzero den => no NaN; they
    # are masked out of layer 2 by e2mask).
    s2_locP = np.zeros(P, dtype=np.int64)
    s2_locP[:n2] = s2_loc
    P2 = (slotmap[:, None] == s2_locP[None, :]).astype(np.float32)

    e2mask = np.full((P, 1), NEG, np.float32)
    e2mask[:n2] = 0.0

    # ---- dense weight prep (host) -------------------------------------------
    W1 = np.asarray(inputs["W1"], np.float32)  # [F, 256]
    a_src1 = np.asarray(inputs["a_src1"], np.float32)  # [4, 64]
    a_dst1 = np.asarray(inputs["a_dst1"], np.float32)
    H1, C = a_src1.shape
    D1 = H1 * C
    rhsA = np.zeros((F, 4 + D1), np.float32)  # [as-cols | W1]
    rhsB = np.zeros((F, 4), np.float32)  # [ad-cols]
    for h in range(H1):
        blk = W1[:, h * C : (h + 1) * C]
        rhsA[:, h] = blk @ a_src1[h]
        rhsB[:, h] = blk @ a_dst1[h]
    rhsA[:, 4:] = W1

    W2 = np.asarray(inputs["W2"], np.float32)  # [256, 64]
    a_src2 = np.asarray(inputs["a_src2"], np.float32)[0]
    a_dst2 = np.asarray(inputs["a_dst2"], np.float32)[0]
    rhs2 = np.zeros((D1, 2 + C), np.float32)  # [ad2 | as2 | W2]
    rhs2[:, 0] = W2 @ a_dst2
    rhs2[:, 1] = W2 @ a_src2
    rhs2[:, 2:] = W2

    b1 = np.asarray(inputs["b1"], np.float32)
    b2 = np.asarray(inputs["b2"], np.float32)
    fc1_b = np.asarray(inputs["fc1_b"], np.float32)
    fc2_b = np.asarray(inputs["fc2_b"], np.float32)
    # zb: all biases the softmax-division deferral must commute past are zero
    zb = not (np.any(b1) or np.any(b2) or np.any(fc1_b))
    zfc2 = not np.any(fc2_b)

    import ml_dtypes as mld

    # hot pack 1: what the first projection matmul needs (bf16)
    pkh = _Packer(mld.bfloat16)
    pkh.add("rhsA", rhsA)
    pkh.add("rhsB", rhsB)
    pkh.add("zero", np.zeros((P, 1), np.float32))
    # hot pack 2: first exp / first aggregation
    pkg = _Packer(mld.bfloat16)
    pkg.add("P2", P2)
    pkg.add("admask", admask)

    # cold pack: layer-2 / MLP constants (bf16)
    pkv = _Packer(mld.bfloat16)
    pkv.add("onescol", np.ones((P, 1), np.float32))
    pkv.add("onesr", np.ones((1, P), np.float32))
    pkv.add("rhs2a", rhs2[:P])
    pkv.add("rhs2b", rhs2[P:])
    pkv.add("e2mask", e2mask)

    pkf = _Packer(np.float32)
    pkf.add("idf1", np.ones((1, 1), np.float32))
    pkf.add("fc1w", np.ascontiguousarray(np.asarray(inputs["fc1_w"], np.float32)))
    pkf.add("fc2w", np.ascontiguousarray(np.asarray(inputs["fc2_w"], np.float32)))
    pkf.add("fc1b", fc1_b[:, None])
    pkf.add("fc2b", fc2_b[None, :])
    if not zb:
        pkf.add("b1bc", np.broadcast_to(b1, (P, D1)))
        pkf.add("b2row", b2[None, :])

    feed = {"xT": xT, "packh": pkh.finish(), "packg": pkg.finish(),
            "packv": pkv.finish(), "packf": pkf.finish()}
    dims = dict(
        F=F, H1=H1, C=C, K=K, zb=zb, zfc2=zfc2,
        slots_h=tuple(sorted(pkh.slots.items())),
        slots_g=tuple(sorted(pkg.slots.items())),
        slots_v=tuple(sorted(pkv.slots.items())),
        slots_f=tuple(sorted(pkf.slots.items())),
    )
    return feed, dims


def _build(dims):
    from concourse import bacc, mybir, tile

    F = dims["F"]          # 128 input features
    H1 = dims["H1"]        # 4 heads, layer 1
    C = dims["C"]          # 64 channels per head
    D1 = H1 * C            # 256
    G1 = 4 + D1            # 260 = [e-pre (4) | h (256)]
    K = dims["K"]
    zb = dims["zb"]
    zfc2 = dims["zfc2"]
    slots_h = dict(dims["slots_h"])
    slots_g = dict(dims["slots_g"])
    slots_v = dict(dims["slots_v"])
    slots_f = dict(dims["slots_f"])
    WH = max(b for _, b in slots_h.values())
    WG = max(b for _, b in slots_g.values())
    WV = max(b for _, b in slots_v.values())
    WF = max(b for _, b in slots_f.values())
    f32 = mybir.dt.float32
    bf16 = mybir.dt.bfloat16

    nc = bacc.Bacc("TRN2", target_bir_lowering=False, debug=False)

    xT_d = nc.declare_dram_parameter("xT", [F, (K + 1) * P], bf16, isOutput=False)
    ph_d = nc.declare_dram_parameter("packh", [P, WH], bf16, isOutput=False)
    pg_d = nc.declare_dram_parameter("packg", [P, WG], bf16, isOutput=False)
    pv_d = nc.declare_dram_parameter("packv", [P, WV], bf16, isOutput=False)
    pf_d = nc.declare_dram_parameter("packf", [P, WF], f32, isOutput=False)
    out_d = nc.declare_dram_parameter("out", [1, 2], f32, isOutput=True)

    MAX = mybir.AluOpType.max
    ADD = mybir.AluOpType.add
    MUL = mybir.AluOpType.mult
    Copy = mybir.ActivationFunctionType.Copy
    Exp = mybir.ActivationFunctionType.Exp
    Relu = mybir.ActivationFunctionType.Relu
    Prelu = mybir.ActivationFunctionType.Prelu

    with tile.TileContext(nc) as tc:
        with (
            tc.tile_pool(name="const", bufs=1) as cp,
            tc.tile_pool(name="work", bufs=6) as wp,
            tc.tile_pool(name="msgp", bufs=5) as mp,
            tc.tile_pool(name="fin", bufs=1) as fp,
            tc.tile_pool(name="pgp", bufs=4, space="PSUM") as pgp,
            tc.tile_pool(name="acc", bufs=1, space="PSUM") as ap_,
            tc.tile_pool(name="ppp", bufs=2, space="PSUM") as pp,
            tc.tile_pool(name="psm", bufs=1, space="PSUM") as sp,
        ):
            # ---------------- inputs into SBUF (3 DMA queues) ----------------
            # sync + scalar trigger earliest: they carry what the first
            # matmuls need.  gpsimd (slowest to start) gets the late chunks.
            ph_s = cp.tile([P, WH], bf16)
            nc.sync.dma_start(ph_s[:], ph_d[:])
            xT_s = cp.tile([F, (K + 1) * P], bf16)
            c1 = min(3, K + 1) * P
            c2 = min(5, K + 1) * P
            nc.scalar.dma_start(xT_s[:, :c1], xT_d[:, :c1])
            nc.sync.dma_start(xT_s[:, c1:c2], xT_d[:, c1:c2])
            if c2 < (K + 1) * P:
                nc.gpsimd.dma_start(xT_s[:, c2:], xT_d[:, c2:])
            pg_s = cp.tile([P, WG], bf16)
            nc.sync.dma_start(pg_s[:], pg_d[:])
            pv_s = cp.tile([P, WV], bf16)
            nc.scalar.dma_start(pv_s[:], pv_d[:])
            pf_s = cp.tile([P, WF], f32)
            nc.scalar.dma_start(pf_s[:], pf_d[:])

            def sh(name, rows=P):
                a, b = slots_h[name]
                return ph_s[:rows, a:b]

            def sg(name, rows=P):
                a, b = slots_g[name]
                return pg_s[:rows, a:b]

            def sv(name, rows=P):
                a, b = slots_v[name]
                return pv_s[:rows, a:b]

            def sf(name, rows=P):
                a, b = slots_f[name]
                return pf_s[:rows, a:b]

            zcol = sh("zero")[:, 0:1]

            from concourse.masks import make_identity

            identb = cp.tile([P, P], bf16)
            make_identity(nc, identb[:])


            # ---------------- layer-1 edge chunks ----------------------------
            # chunk k: project 128 edge slots -> [as+ad | h], then
            # pe = exp(prelu(e) + mask); msg = [h*pe | pe];
            # agg[q, :] += P2 @ msg  (partition merge + L2 gather fused)
            agg = ap_.tile([P, G1], f32, tag="agg")
            for k in range(K):
                pg = pgp.tile([P, G1], f32, tag="pg")
                nc.tensor.matmul(
                    out=pg[:], lhsT=xT_s[:, (k + 1) * P : (k + 2) * P],
                    rhs=sh("rhsA"), start=True, stop=False,
                )
                nc.tensor.matmul(
                    out=pg[:, 0:4], lhsT=xT_s[:, :P], rhs=sh("rhsB"),
                    start=False, stop=True, skip_group_check=True,
                )
                el = wp.tile([P, 4], f32, tag="el")
                nc.scalar.activation(el[:], pg[:, 0:4], Prelu, bias=zcol,
                                     alpha=0.2)
                msg = mp.tile([P, G1], bf16, tag="msg")
                nc.scalar.activation(
                    msg[:, D1:], el[:], Exp, bias=sg("admask")[:, k : k + 1]
                )
                nc.vector.tensor_tensor(
                    out=msg[:, :D1].rearrange("p (h c) -> p h c", c=C),
                    in0=pg[:, 4:].rearrange("p (h c) -> p h c", c=C),
                    in1=msg[:, D1:].rearrange("p (h o) -> p h o", o=1)
                    .to_broadcast([P, H1, C]),
                    op=MUL,
                )
                nc.tensor.matmul(
                    out=agg[:], lhsT=sg("P2"), rhs=msg[:],
                    start=(k == 0), stop=(k == K - 1),
                )

            # -------- layer-1 finalize: h1g = relu(num * (1/den)) per head ---
            # stage agg into SBUF once: ScalarE and VectorE cannot touch the
            # same PSUM bank concurrently, but from SBUF they run in parallel
            # (and the DVE ops get the 2x single-source mode)
            aggS = fp.tile([P, G1], f32)
            nc.scalar.copy(aggS[:], agg[:])
            rec = fp.tile([P, H1], f32)
            nc.vector.reciprocal(rec[:], aggS[:, D1:])
            h1gL = fp.tile([P, P], bf16)
            h1gR = fp.tile([P, P], bf16)
            halves = [h1gL, h1gR]
            if zb:
                for h in range(H1):
                    ds = slice(h * C, (h + 1) * C)
                    hs = slice((h % 2) * C, (h % 2 + 1) * C)
                    if h < 3:
                        nc.vector.tensor_scalar(
                            halves[h // 2][:, hs], aggS[:, ds],
                            rec[:, h : h + 1], 0.0, MUL, MAX,
                        )
                    else:
                        nc.scalar.activation(
                            h1gR[:, hs], aggS[:, ds], Relu, bias=zcol,
                            scale=rec[:, h : h + 1],
                        )
            else:
                h1f = fp.tile([P, D1], f32)
                for h in range(H1):
                    ds = slice(h * C, (h + 1) * C)
                    nc.vector.tensor_scalar(
                        h1f[:, ds], aggS[:, ds], rec[:, h : h + 1], None, MUL
                    )
                h1b = fp.tile([P, D1], f32)
                nc.vector.tensor_add(h1b[:], h1f[:], sf("b1bc"))
                nc.scalar.activation(h1gL[:], h1b[:, :P], Relu)
                nc.scalar.activation(h1gR[:], h1b[:, P:], Relu)

            # ---------------- layer-2 projection ------------------------------
            # pg2[q, :] = h1g[q] @ [W2ad2 | W2as2 | W2]; ad2(ball) via q=0.
            # both transposes first (two PSUM banks) so neither waits in
            # the TensorE FIFO behind the first half-contraction
            h1T = []
            for c in range(2):
                ptr = pp.tile([P, P], bf16, tag="tr")
                nc.tensor.transpose(
                    out=ptr[:], in_=halves[c][:], identity=identb[:],
                )
                t = fp.tile([P, P], bf16)
                nc.vector.tensor_copy(t[:], ptr[:])
                h1T.append(t)
            pg2 = ap_.tile([P, 2 + C], f32, tag="agg")
            for c in range(2):
                nc.tensor.matmul(
                    out=pg2[:], lhsT=h1T[c][:], rhs=sv(f"rhs2{'ab'[c]}"),
                    start=(c == 0), stop=False, skip_group_check=(c == 1),
                )
                # ball ad2 broadcast: stride-0 lhsT replicates h1T column 0
                # (the ball) across all 128 output slots of the e2 column
                nc.tensor.matmul(
                    out=pg2[:, 1:2],
                    lhsT=h1T[c][:, 0:1].to_broadcast([P, P]),
                    rhs=sv(f"rhs2{'ab'[c]}")[:, 0:1],
                    start=False, stop=(c == 1), skip_group_check=True,
                )
            # ---------------- layer-2 softmax + aggregate ---------------------
            el2 = fp.tile([P, 1], f32)
            nc.scalar.activation(
                el2[:], pg2[:, 1:2], Prelu, bias=sv("e2mask"), alpha=0.2
            )
            rhs2t = fp.tile([P, C + 1], bf16)
            nc.scalar.activation(rhs2t[:, C:], el2[:], Exp, bias=zcol)
            nc.vector.tensor_tensor(
                out=rhs2t[:, :C], in0=pg2[:, 2:],
                in1=rhs2t[:, C:].to_broadcast([P, C]), op=MUL,
            )
            # ---------------- ball column + MLP (division deferred) ----------
            bcol = fp.tile([C, 1], f32)
            rec2 = fp.tile([1, 1], f32)
            if zb:
                # transposed aggregate gives the unnormalized ball column;
                # the row aggregate (partition 0) feeds the reciprocal
                agg2c = pp.tile([C + 1, 1], f32, tag="tr")
                nc.tensor.matmul(
                    out=agg2c[:], lhsT=rhs2t[:], rhs=sv("onescol"),
                    start=True, stop=True,
                )
                agg2 = sp.tile([1, C + 1], f32, tag="mm")
                nc.tensor.matmul(
                    out=agg2[:, C:], lhsT=sv("onescol"), rhs=rhs2t[:, C:],
                    start=True, stop=True,
                )
                nc.vector.tensor_scalar_max(bcol[:], agg2c[:C, 0:1], 0.0)
                nc.vector.reciprocal(rec2[:], agg2[0:1, C : C + 1])
            else:
                agg2 = sp.tile([1, C + 1], f32, tag="mm")
                nc.tensor.matmul(
                    out=agg2[:], lhsT=sv("onescol"), rhs=rhs2t[:],
                    start=True, stop=True,
                )
                nc.vector.reciprocal(rec2[:], agg2[0:1, C : C + 1])
                bb = fp.tile([1, C], f32)
                nc.vector.tensor_scalar(
                    bb[:], agg2[0:1, :C], rec2[0:1, 0:1], None, MUL
                )
                bb2 = fp.tile([1, C], f32)
                nc.vector.tensor_add(bb2[:], bb[:], sf("b2row", 1))
                brow = fp.tile([1, C], f32)
                nc.scalar.activation(brow[:], bb2[:], Relu)
                ptb = sp.tile([C, 1], f32, tag="mm")
                nc.tensor.transpose(
                    out=ptb[:], in_=brow[:], identity=sf("idf1", 1)[0:1, 0:1]
                )
                nc.vector.tensor_copy(bcol[:], ptb[:])
            z = sp.tile([C // 2, 1], f32, tag="mm")
            nc.tensor.matmul(
                out=z[:], lhsT=sf("fc1w", C), rhs=bcol[:], start=True, stop=True
            )
            zr = fp.tile([C // 2, 1], f32)
            if zb:
                nc.vector.tensor_scalar_max(zr[:], z[:], 0.0)
            else:
                nc.scalar.activation(zr[:], z[:], Relu, bias=sf("fc1b", C // 2))
            fin2 = sp.tile([1, 2], f32, tag="mm")
            nc.tensor.matmul(
                out=fin2[:], lhsT=zr[:], rhs=sf("fc2w", C // 2),
                start=True, stop=True,
            )
            osb = fp.tile([1, 2], f32)
            if zb:
                nc.vector.tensor_scalar(
                    osb[:], fin2[0:1, :], rec2[0:1, 0:1], None, MUL
                )
                if not zfc2:
                    osb2 = fp.tile([1, 2], f32)
                    nc.vector.tensor_add(osb2[:], osb[:], sf("fc2b", 1))
                    osb = osb2
            else:
                # division already applied before the MLP in this path
                osb2 = fp.tile([1, 2], f32)
                nc.vector.tensor_add(osb2[:], fin2[0:1, :], sf("fc2b", 1))
                osb = osb2
            nc.sync.dma_start(out_d[:], osb[:], single_packet=True)

    nc.compile()
    return nc


def kernel(**inputs):
    from concourse.bass_utils import run_bass_kernel_spmd

    feed, dims = _host_preprocess(inputs)
    key = (dims["F"], dims["H1"], dims["C"], dims["K"], dims["zb"], dims["zfc2"])
    if key not in _CACHE:
        _CACHE[key] = _build(dims)
    nc = _CACHE[key]

    n_cores = 8
    in_maps = [dict(feed) for _ in range(n_cores)]
    res = run_bass_kernel_spmd(nc, in_maps, core_ids=list(range(n_cores)))
    out = np.asarray(res.results[0]["out"], dtype=np.float32).reshape(2)
    return out


# revision 56
# speedup vs baseline: 1.0231x; 1.0231x over previous
"""Trainium2 Bass kernel for nn_BallPredictorGNN.

The reference model is a 2-layer GAT over (N=20000, E=640000) followed by an
MLP applied to the LAST node only ("ball") -- the output is a single [2]
vector.  Only the ball's 2-hop dependency cone matters:

  layer 2 aggregates at the ball node only            (~33 in-edges)
  layer 1 aggregates at the ball's in-neighbours S2   (~26 nodes, ~850 edges)
  x @ W1 is needed for the sources of those edges

Host side (pure data routing): extract the cone and lay layer-1 edges out on
a [128 partitions x K chunks] grid; each partition serves one destination
node (high-degree destinations get several partitions).  Source features are
replicated per edge-slot into the xTs operand; destination features go into
a per-partition xTd operand, so a pair of accumulating TensorE matmuls
produces per-edge rows [as+ad | h] directly -- no on-device index math.

Device side: per chunk, ScalarE computes pe = exp(prelu(e)+mask) (both in
the exp table set, mask folds padding to exp(-1e30)=0), VectorE forms
msg = [h*pe | pe], and one TensorE matmul against a host-built one-hot P2
merges partition groups AND gathers layer-2 edge slots in one shot
(agg[q] = sum over layer-1 edges of dst(q)).  Layer 2 then runs fully
on-chip on the [128 edge-slot, .] layout; the softmax division is deferred
through the (relu-positive-homogeneous, zero-bias) MLP and applied as a
single fused Copy(scale=1/den) at the very end.

The same program is replicated SPMD on all 8 NeuronCores (the cone is tiny,
so replication beats sharding + collectives); core 0's output is returned.
"""

import numpy as np

P = 128
NEG = np.float32(-1e30)
_CACHE = {}


def _ceil(a, b):
    return -(-a // b)


class _Packer:
    """Pack many small [p, w] operands into one [128, W] array, column-wise."""

    def __init__(self, dtype):
        self.cols = []
        self.pos = 0
        self.slots = {}
        self.dtype = dtype

    def add(self, name, arr):
        arr = np.asarray(arr, dtype=np.float32)
        p, w = arr.shape
        full = np.zeros((P, w), np.float32)
        full[:p] = arr
        self.cols.append(full)
        self.slots[name] = (self.pos, self.pos + w)
        self.pos += w

    def finish(self):
        return np.ascontiguousarray(
            np.concatenate(self.cols, axis=1).astype(self.dtype)
        )


def _host_preprocess(inputs):
    import ml_dtypes

    x = np.asarray(inputs["x"], dtype=np.float32)
    ei = np.asarray(inputs["edge_index"]).astype(np.int64)
    N, F = x.shape
    ball = N - 1
    src, dst = ei[0], ei[1]

    # ---- layer-2 edges into the ball: self loop FIRST (slot q=0) ------------
    e2s = np.concatenate([[ball], src[dst == ball]]).astype(np.int64)
    n2 = len(e2s)
    assert n2 <= P, f"ball in-neighbourhood too large: {n2}"
    uniq = np.unique(e2s)
    S2 = np.concatenate([[ball], uniq[uniq != ball]]).astype(np.int64)
    m2 = len(S2)
    loc2 = np.full(N, -1, dtype=np.int64)
    loc2[S2] = np.arange(m2)
    s2_loc = loc2[e2s]  # [n2], s2_loc[0] == 0 (ball)

    # ---- layer-1 edge grid: [partition, chunk] ------------------------------
    in_S2 = np.zeros(N, dtype=bool)
    in_S2[S2] = True
    sel1 = in_S2[dst]
    l1s, l1d = src[sel1], loc2[dst[sel1]]
    by_dst = [[v] for v in S2]  # reference adds a self loop to every node
    for s, d in zip(l1s, l1d):
        by_dst[d].append(s)

    K = 2
    while sum(_ceil(len(g), K) for g in by_dst) > P:
        K += 1
    nparts = [_ceil(len(g), K) for g in by_dst]
    assert sum(nparts) <= P

    grid_src = np.zeros((P, K), dtype=np.int64)
    grid_valid = np.zeros((P, K), dtype=bool)
    slotmap = np.full(P, -1, dtype=np.int64)  # partition -> S2 index
    p = 0
    for sidx in range(m2):
        g = by_dst[sidx]
        for gi in range(nparts[sidx]):
            chunk = g[gi * K : (gi + 1) * K]
            for j, s in enumerate(chunk):
                grid_src[p, j] = s
                grid_valid[p, j] = True
            slotmap[p] = sidx
            p += 1

    # xT: [F, (K+1)*128].  Block 0 = destination features per partition;
    # block k+1, column q = x[grid_src[q, k]] (zeroed if pad).
    xd = x[S2[np.maximum(slotmap, 0)]]
    xd[slotmap < 0] = 0
    xg = x[grid_src.T.reshape(-1)]
    xg[~grid_valid.T.reshape(-1)] = 0
    xT = np.ascontiguousarray(
        np.concatenate([xd, xg], axis=0).T.astype(ml_dtypes.bfloat16)
    )

    admask = np.where(grid_valid, np.float32(0), NEG).astype(np.float32)

    # P2: layer-1 aggregation + layer-2 gather one-hot.
    # P2[p, q] = 1 iff partition p serves the dst node of layer-2 edge slot q.
    # Padding slots q>=n2 reuse the ball column (nonzero den => no NaN; they
    # are masked out of layer 2 by e2mask).
    s2_locP = np.zeros(P, dtype=np.int64)
    s2_locP[:n2] = s2_loc
    P2 = (slotmap[:, None] == s2_locP[None, :]).astype(np.float32)

    e2mask = np.full((P, 1), NEG, np.float32)
    e2mask[:n2] = 0.0

    # ---- dense weight prep (host) -------------------------------------------
    W1 = np.asarray(inputs["W1"], np.float32)  # [F, 256]
    a_src1 = np.asarray(inputs["a_src1"], np.float32)  # [4, 64]
    a_dst1 = np.asarray(inputs["a_dst1"], np.float32)
    H1, C = a_src1.shape
    D1 = H1 * C
    rhsA = np.zeros((F, 4 + D1), np.float32)  # [as-cols | W1]
    rhsB = np.zeros((F, 4), np.float32)  # [ad-cols]
    for h in range(H1):
        blk = W1[:, h * C : (h + 1) * C]
        rhsA[:, h] = blk @ a_src1[h]
        rhsB[:, h] = blk @ a_dst1[h]
    rhsA[:, 4:] = W1

    W2 = np.asarray(inputs["W2"], np.float32)  # [256, 64]
    a_src2 = np.asarray(inputs["a_src2"], np.float32)[0]
    a_dst2 = np.asarray(inputs["a_dst2"], np.float32)[0]
    rhs2 = np.zeros((D1, 2 + C), np.float32)  # [ad2 | as2 | W2]
    rhs2[:, 0] = W2 @ a_dst2
    rhs2[:, 1] = W2 @ a_src2
    rhs2[:, 2:] = W2

    b1 = np.asarray(inputs["b1"], np.float32)
    b2 = np.asarray(inputs["b2"], np.float32)
    fc1_b = np.asarray(inputs["fc1_b"], np.float32)
    fc2_b = np.asarray(inputs["fc2_b"], np.float32)
    # zb: all biases the softmax-division deferral must commute past are zero
    zb = not (np.any(b1) or np.any(b2) or np.any(fc1_b))
    zfc2 = not np.any(fc2_b)

    import ml_dtypes as mld

    # hot pack 1: what the first projection matmul needs (bf16)
    pkh = _Packer(mld.bfloat16)
    pkh.add("rhsA", rhsA)
    pkh.add("rhsB", rhsB)
    pkh.add("zero", np.zeros((P, 1), np.float32))
    # hot pack 2: first exp / first aggregation
    pkg = _Packer(mld.bfloat16)
    pkg.add("P2", P2)
    pkg.add("admask", admask)

    # cold pack: layer-2 / MLP constants (bf16)
    pkv = _Packer(mld.bfloat16)
    pkv.add("onescol", np.ones((P, 1), np.float32))
    pkv.add("onesr", np.ones((1, P), np.float32))
    pkv.add("rhs2a", rhs2[:P])
    pkv.add("rhs2b", rhs2[P:])
    pkv.add("e2mask", e2mask)

    pkf = _Packer(np.float32)
    pkf.add("idf1", np.ones((1, 1), np.float32))
    pkf.add("fc1w", np.ascontiguousarray(np.asarray(inputs["fc1_w"], np.float32)))
    pkf.add("fc2w", np.ascontiguousarray(np.asarray(inputs["fc2_w"], np.float32)))
    pkf.add("fc1b", fc1_b[:, None])
    pkf.add("fc2b", fc2_b[None, :])
    if not zb:
        pkf.add("b1bc", np.broadcast_to(b1, (P, D1)))
        pkf.add("b2row", b2[None, :])

    feed = {"xT": xT, "packh": pkh.finish(), "packg": pkg.finish(),
            "packv": pkv.finish(), "packf": pkf.finish()}
    dims = dict(
        F=F, H1=H1, C=C, K=K, zb=zb, zfc2=zfc2,
        slots_h=tuple(sorted(pkh.slots.items())),
        slots_g=tuple(sorted(pkg.slots.items())),
        slots_v=tuple(sorted(pkv.slots.items())),
        slots_f=tuple(sorted(pkf.slots.items())),
    )
    return feed, dims


def _build(dims):
    from concourse import bacc, mybir, tile

    F = dims["F"]          # 128 input features
    H1 = dims["H1"]        # 4 heads, layer 1
    C = dims["C"]          # 64 channels per head
    D1 = H1 * C            # 256
    G1 = 4 + D1            # 260 = [e-pre (4) | h (256)]
    K = dims["K"]
    zb = dims["zb"]
    zfc2 = dims["zfc2"]
    slots_h = dict(dims["slots_h"])
    slots_g = dict(dims["slots_g"])
    slots_v = dict(dims["slots_v"])
    slots_f = dict(dims["slots_f"])
    WH = max(b for _, b in slots_h.values())
    WG = max(b for _, b in slots_g.values())
    WV = max(b for _, b in slots_v.values())
    WF = max(b for _, b in slots_f.values())
    f32 = mybir.dt.float32
    bf16 = mybir.dt.bfloat16

    nc = bacc.Bacc("TRN2", target_bir_lowering=False, debug=False)

    xT_d = nc.declare_dram_parameter("xT", [F, (K + 1) * P], bf16, isOutput=False)
    ph_d = nc.declare_dram_parameter("packh", [P, WH], bf16, isOutput=False)
    pg_d = nc.declare_dram_parameter("packg", [P, WG], bf16, isOutput=False)
    pv_d = nc.declare_dram_parameter("packv", [P, WV], bf16, isOutput=False)
    pf_d = nc.declare_dram_parameter("packf", [P, WF], f32, isOutput=False)
    out_d = nc.declare_dram_parameter("out", [1, 2], f32, isOutput=True)

    MAX = mybir.AluOpType.max
    ADD = mybir.AluOpType.add
    MUL = mybir.AluOpType.mult
    Copy = mybir.ActivationFunctionType.Copy
    Exp = mybir.ActivationFunctionType.Exp
    Relu = mybir.ActivationFunctionType.Relu
    Prelu = mybir.ActivationFunctionType.Prelu

    with tile.TileContext(nc) as tc:
        with (
            tc.tile_pool(name="const", bufs=1) as cp,
            tc.tile_pool(name="work", bufs=6) as wp,
            tc.tile_pool(name="msgp", bufs=5) as mp,
            tc.tile_pool(name="fin", bufs=1) as fp,
            tc.tile_pool(name="pgp", bufs=5, space="PSUM") as pgp,
            tc.tile_pool(name="acc", bufs=1, space="PSUM") as ap_,
            tc.tile_pool(name="ppp", bufs=1, space="PSUM") as pp,
            tc.tile_pool(name="psm", bufs=1, space="PSUM") as sp,
        ):
            # ---------------- inputs into SBUF (3 DMA queues) ----------------
            # sync + scalar trigger earliest: they carry what the first
            # matmuls need.  gpsimd (slowest to start) gets the late chunks.
            ph_s = cp.tile([P, WH], bf16)
            nc.sync.dma_start(ph_s[:], ph_d[:])
            xT_s = cp.tile([F, (K + 1) * P], bf16)
            c1 = min(3, K + 1) * P
            c2 = min(5, K + 1) * P
            nc.scalar.dma_start(xT_s[:, :c1], xT_d[:, :c1])
            nc.sync.dma_start(xT_s[:, c1:c2], xT_d[:, c1:c2])
            if c2 < (K + 1) * P:
                nc.gpsimd.dma_start(xT_s[:, c2:], xT_d[:, c2:])
            pg_s = cp.tile([P, WG], bf16)
            nc.sync.dma_start(pg_s[:], pg_d[:])
            pv_s = cp.tile([P, WV], bf16)
            nc.scalar.dma_start(pv_s[:], pv_d[:])
            pf_s = cp.tile([P, WF], f32)
            nc.scalar.dma_start(pf_s[:], pf_d[:])

            def sh(name, rows=P):
                a, b = slots_h[name]
                return ph_s[:rows, a:b]

            def sg(name, rows=P):
                a, b = slots_g[name]
                return pg_s[:rows, a:b]

            def sv(name, rows=P):
                a, b = slots_v[name]
                return pv_s[:rows, a:b]

            def sf(name, rows=P):
                a, b = slots_f[name]
                return pf_s[:rows, a:b]

            zcol = sh("zero")[:, 0:1]

            from concourse.masks import make_identity

            identb = cp.tile([P, P], bf16)
            make_identity(nc, identb[:])


            # ---------------- layer-1 edge chunks ----------------------------
            # chunk k: project 128 edge slots -> [as+ad | h], then
            # pe = exp(prelu(e) + mask); msg = [h*pe | pe];
            # agg[q, :] += P2 @ msg  (partition merge + L2 gather fused)
            agg = ap_.tile([P, G1], f32, tag="agg")
            for k in range(K):
                pg = pgp.tile([P, G1], f32, tag="pg")
                nc.tensor.matmul(
                    out=pg[:], lhsT=xT_s[:, (k + 1) * P : (k + 2) * P],
                    rhs=sh("rhsA"), start=True, stop=False,
                )
                nc.tensor.matmul(
                    out=pg[:, 0:4], lhsT=xT_s[:, :P], rhs=sh("rhsB"),
                    start=False, stop=True, skip_group_check=True,
                )
                el = wp.tile([P, 4], f32, tag="el")
                nc.scalar.activation(el[:], pg[:, 0:4], Prelu, bias=zcol,
                                     alpha=0.2)
                msg = mp.tile([P, G1], bf16, tag="msg")
                nc.scalar.activation(
                    msg[:, D1:], el[:], Exp, bias=sg("admask")[:, k : k + 1]
                )
                nc.vector.tensor_tensor(
                    out=msg[:, :D1].rearrange("p (h c) -> p h c", c=C),
                    in0=pg[:, 4:].rearrange("p (h c) -> p h c", c=C),
                    in1=msg[:, D1:].rearrange("p (h o) -> p h o", o=1)
                    .to_broadcast([P, H1, C]),
                    op=MUL,
                )
                nc.tensor.matmul(
                    out=agg[:], lhsT=sg("P2"), rhs=msg[:],
                    start=(k == 0), stop=(k == K - 1),
                )

            # -------- layer-1 finalize: h1g = relu(num * (1/den)) per head ---
            # stage agg into SBUF once: ScalarE and VectorE cannot touch the
            # same PSUM bank concurrently, but from SBUF they run in parallel
            # (and the DVE ops get the 2x single-source mode)
            aggS = fp.tile([P, G1], f32)
            nc.scalar.copy(aggS[:], agg[:])
            rec = fp.tile([P, H1], f32)
            nc.vector.reciprocal(rec[:], aggS[:, D1:])
            h1gL = fp.tile([P, P], bf16)
            h1gR = fp.tile([P, P], bf16)
            halves = [h1gL, h1gR]
            if zb:
                for h in range(H1):
                    ds = slice(h * C, (h + 1) * C)
                    hs = slice((h % 2) * C, (h % 2 + 1) * C)
                    if h < 3:
                        nc.vector.tensor_scalar(
                            halves[h // 2][:, hs], aggS[:, ds],
                            rec[:, h : h + 1], 0.0, MUL, MAX,
                        )
                    else:
                        nc.scalar.activation(
                            h1gR[:, hs], aggS[:, ds], Relu, bias=zcol,
                            scale=rec[:, h : h + 1],
                        )
            else:
                h1f = fp.tile([P, D1], f32)
                for h in range(H1):
                    ds = slice(h * C, (h + 1) * C)
                    nc.vector.tensor_scalar(
                        h1f[:, ds], aggS[:, ds], rec[:, h : h + 1], None, MUL
                    )
                h1b = fp.tile([P, D1], f32)
                nc.vector.tensor_add(h1b[:], h1f[:], sf("b1bc"))
                nc.scalar.activation(h1gL[:], h1b[:, :P], Relu)
                nc.scalar.activation(h1gR[:], h1b[:, P:], Relu)

            # ---------------- layer-2 projection ------------------------------
            # pg2[q, :] = h1g[q] @ [W2ad2 | W2as2 | W2]; ad2(ball) via q=0.
            h1T = []
            pg2 = ap_.tile([P, 2 + C], f32, tag="agg")
            for c in range(2):
                ptr = pp.tile([P, P], bf16, tag="tr")
                nc.tensor.transpose(
                    out=ptr[:], in_=halves[c][:], identity=identb[:],
                )
                t = fp.tile([P, P], bf16)
                nc.vector.tensor_copy(t[:], ptr[:])
                h1T.append(t)
                nc.tensor.matmul(
                    out=pg2[:], lhsT=t[:], rhs=sv(f"rhs2{'ab'[c]}"),
                    start=(c == 0), stop=False, skip_group_check=(c == 1),
                )
                # ball ad2 broadcast: stride-0 lhsT replicates h1T column 0
                # (the ball) across all 128 output slots of the e2 column
                nc.tensor.matmul(
                    out=pg2[:, 1:2],
                    lhsT=t[:, 0:1].to_broadcast([P, P]),
                    rhs=sv(f"rhs2{'ab'[c]}")[:, 0:1],
                    start=False, stop=(c == 1), skip_group_check=True,
                )
            # ---------------- layer-2 softmax + aggregate ---------------------
            el2 = fp.tile([P, 1], f32)
            nc.scalar.activation(
                el2[:], pg2[:, 1:2], Prelu, bias=sv("e2mask"), alpha=0.2
            )
            rhs2t = fp.tile([P, C + 1], bf16)
            nc.scalar.activation(rhs2t[:, C:], el2[:], Exp, bias=zcol)
            nc.vector.tensor_tensor(
                out=rhs2t[:, :C], in0=pg2[:, 2:],
                in1=rhs2t[:, C:].to_broadcast([P, C]), op=MUL,
            )
            # ---------------- ball column + MLP (division deferred) ----------
            bcol = fp.tile([C, 1], f32)
            rec2 = fp.tile([1, 1], f32)
            if zb:
                # transposed aggregate gives the unnormalized ball column;
                # the row aggregate (partition 0) feeds the reciprocal
                agg2c = pp.tile([C + 1, 1], f32, tag="tr")
                nc.tensor.matmul(
                    out=agg2c[:], lhsT=rhs2t[:], rhs=sv("onescol"),
                    start=True, stop=True,
                )
                agg2 = sp.tile([1, C + 1], f32, tag="mm")
                nc.tensor.matmul(
                    out=agg2[:, C:], lhsT=sv("onescol"), rhs=rhs2t[:, C:],
                    start=True, stop=True,
                )
                nc.vector.tensor_scalar_max(bcol[:], agg2c[:C, 0:1], 0.0)
                nc.vector.reciprocal(rec2[:], agg2[0:1, C : C + 1])
            else:
                agg2 = sp.tile([1, C + 1], f32, tag="mm")
                nc.tensor.matmul(
                    out=agg2[:], lhsT=sv("onescol"), rhs=rhs2t[:],
                    start=True, stop=True,
                )
                nc.vector.reciprocal(rec2[:], agg2[0:1, C : C + 1])
                bb = fp.tile([1, C], f32)
                nc.vector.tensor_scalar(
                    bb[:], agg2[0:1, :C], rec2[0:1, 0:1], None, MUL
                )
                bb2 = fp.tile([1, C], f32)
                nc.vector.tensor_add(bb2[:], bb[:], sf("b2row", 1))
                brow = fp.tile([1, C], f32)
                nc.scalar.activation(brow[:], bb2[:], Relu)
                ptb = sp.tile([C, 1], f32, tag="mm")
                nc.tensor.transpose(
                    out=ptb[:], in_=brow[:], identity=sf("idf1", 1)[0:1, 0:1]
                )
                nc.vector.tensor_copy(bcol[:], ptb[:])
            z = sp.tile([C // 2, 1], f32, tag="mm")
            nc.tensor.matmul(
                out=z[:], lhsT=sf("fc1w", C), rhs=bcol[:], start=True, stop=True
            )
            zr = fp.tile([C // 2, 1], f32)
            if zb:
                nc.vector.tensor_scalar_max(zr[:], z[:], 0.0)
            else:
                nc.scalar.activation(zr[:], z[:], Relu, bias=sf("fc1b", C // 2))
            fin2 = sp.tile([1, 2], f32, tag="mm")
            nc.tensor.matmul(
                out=fin2[:], lhsT=zr[:], rhs=sf("fc2w", C // 2),
                start=True, stop=True,
            )
            osb = fp.tile([1, 2], f32)
            if zb:
                nc.vector.tensor_scalar(
                    osb[:], fin2[0:1, :], rec2[0:1, 0:1], None, MUL
                )
                if not zfc2:
                    osb2 = fp.tile([1, 2], f32)
                    nc.vector.tensor_add(osb2[:], osb[:], sf("fc2b", 1))
                    osb = osb2
            else:
                # division already applied before the MLP in this path
                osb2 = fp.tile([1, 2], f32)
                nc.vector.tensor_add(osb2[:], fin2[0:1, :], sf("fc2b", 1))
                osb = osb2
            nc.sync.dma_start(out_d[:], osb[:], single_packet=True)

    nc.compile()
    return nc


def kernel(**inputs):
    from concourse.bass_utils import run_bass_kernel_spmd

    feed, dims = _host_preprocess(inputs)
    key = (dims["F"], dims["H1"], dims["C"], dims["K"], dims["zb"], dims["zfc2"])
    if key not in _CACHE:
        _CACHE[key] = _build(dims)
    nc = _CACHE[key]

    n_cores = 8
    in_maps = [dict(feed) for _ in range(n_cores)]
    res = run_bass_kernel_spmd(nc, in_maps, core_ids=list(range(n_cores)))
    out = np.asarray(res.results[0]["out"], dtype=np.float32).reshape(2)
    return out
